# revision 28
# baseline (speedup 1.0000x reference)
"""GQA causal attention (B=2, S=2048, 32 Q heads / 8 KV heads, d=128) on 8 TRN2 cores.

Sharding: core c -> batch c//4, head-block c%4 (8 Q heads, 2 KV heads each).
Host side: inputs cast to bf16 and Q/K pre-transposed to (d, seq) layout, so
the device does no transposes and every DMA is a plain contiguous load.
Per-core kernel: S^T = K @ Q^T on the tensor engine (bf16, fp32 PSUM);
exp without max-subtraction (scores are O(6)); row sums via a ones-column
appended to V; causal handled by skipping k>q tiles, trimming diagonal-chunk
matmuls/exp to the valid q-range, and a triangular mask on diagonal blocks.
Softmax normalization and the output stay fp32.
"""

import sys

for p in ("/opt/trn_rl_repo", "/root/.axon_site/_ro/trn_rl_repo"):
    if p not in sys.path:
        sys.path.insert(0, p)

import numpy as np

import concourse.bass as bass
import concourse.mybir as mybir
import concourse.tile as tile
from concourse import bacc
from concourse.bass_utils import run_bass_kernel_spmd
from concourse.masks import make_upper_triangular

S = 2048            # sequence length
D = 128             # head size
NQH = 8             # query heads per core
NKVH = 2            # kv heads per core
GROUP = 4           # query heads per kv head
SCALE = 1.0 / np.sqrt(128.0)
NQT = S // D        # 16 q/k tiles of 128 rows
QB = 512            # q-block (4 tiles)
NQB = S // QB       # 4 q-blocks
F32 = mybir.dt.float32
BF16 = mybir.dt.bfloat16


def build_nc():
    nc = bacc.Bacc()
    # queryT/keyT are host-pretransposed to (d, seq); value is natural (seq, d)
    q_ext = nc.declare_dram_parameter("queryT", [NQH * D, S], BF16, isOutput=False)
    k_ext = nc.declare_dram_parameter("keyT", [NKVH * D, S], BF16, isOutput=False)
    v_ext = nc.declare_dram_parameter("value", [S, NKVH * D], BF16, isOutput=False)
    o_ext = nc.declare_dram_parameter("out", [S, NQH * D], F32, isOutput=True)

    with tile.TileContext(nc) as tc:
        with (
            tc.tile_pool(name="singles", bufs=1) as singles,
            tc.tile_pool(name="qt", bufs=3) as qt_pool,
            tc.tile_pool(name="pt", bufs=4) as pt_pool,
            tc.tile_pool(name="small", bufs=4) as small,
            tc.tile_pool(name="ost", bufs=4) as ost_pool,
            tc.tile_pool(name="st", bufs=2, space="PSUM") as st_pool,
            tc.tile_pool(name="oa", bufs=2, space="PSUM") as oa_pool,
        ):
            umask = singles.tile([D, D], BF16)  # umask[k, q] = 1.0 if k <= q else 0
            make_upper_triangular(nc, umask, val=1.0, diag=True)

            vv_view = v_ext[:].rearrange("(ki k) (g d) -> k ki g d", k=D, d=D)
            o_view = o_ext[:].rearrange("(qb qs r) (h d) -> r qb qs h d", r=D, qs=4, d=D)

            # K^T per kv head: [d, k_global], plain load (host-transposed).
            # First chunk of g=0 lands first so matmuls can start early.
            kts = []
            for g in range(NKVH):
                kt = singles.tile([D, S], BF16, tag=f"kt{g}")
                kts.append(kt)
            nc.sync.dma_start(out=kts[0][:, 0:QB], in_=k_ext[0:D, 0:QB])
            qt0 = qt_pool.tile([D, S], BF16, tag="qt")
            for c in range(4):
                nc.sync.dma_start(
                    out=qt0[:, c * QB:(c + 1) * QB],
                    in_=q_ext[0:D, c * QB:(c + 1) * QB],
                )
            for g in range(NKVH):
                for c in range(4):
                    if g == 0 and c == 0:
                        continue
                    nc.sync.dma_start(
                        out=kts[g][:, c * QB:(c + 1) * QB],
                        in_=k_ext[g * D:(g + 1) * D, c * QB:(c + 1) * QB],
                    )

            # V per kv head, natural layout + ones column: [k_off, ki, d|1].
            # Loaded via the gpsimd queue to keep the sync queue free for Q/K.
            vas = []
            for g in range(NKVH):
                va = singles.tile([D, NQT, D + 1], BF16, tag=f"va{g}")
                vas.append(va)
                nc.vector.memset(va[:, :, D], 1.0)
                nc.gpsimd.dma_start(out=va[:, :, 0:D], in_=vv_view[:, :, g, :])

            def emit_scores(h, qb, qt):
                """S^T matmuls + exp + diagonal mask for one (head, q-block).
                Returns the P^T tile."""
                g = h // GROUP
                kt = kts[g]
                nki = 4 * qb + 4  # causal: k tiles 0 .. 4qb+3
                qts = qt[:, qb * QB:(qb + 1) * QB]
                pt = pt_pool.tile([D, NQT, QB], BF16, tag="pt")
                CH = 3  # k-tiles per PSUM chunk (3 banks x 2 bufs + 2 oa = 8)
                for c in range((nki + CH - 1) // CH):
                    k0 = c * CH
                    nk = min(CH, nki - k0)
                    st = st_pool.tile([D, CH, QB], F32, tag="st")
                    for j in range(nk):
                        ki = k0 + j
                        s = ki - 4 * qb  # >=0 only in the 4 diagonal k-tiles
                        q0 = max(s, 0) * D  # valid q-range starts at tile s
                        nc.tensor.matmul(
                            st[:, j, q0:QB],
                            kt[:, ki * D:(ki + 1) * D], qts[:, q0:QB],
                            start=True, stop=True,
                        )
                    # exp only the valid q-range of the chunk (q >= the
                    # first k-tile's diagonal); unseen regions are never read
                    qc = max(k0 - 4 * qb, 0) * D
                    nc.scalar.activation(
                        pt[:, k0:k0 + nk, qc:QB], st[:, 0:nk, qc:QB],
                        mybir.ActivationFunctionType.Exp,
                        scale=float(SCALE),
                    )
                # causal fixup: mask the diagonal blocks (ki == qi).
                # Blocks with ki > qi are never read by the PV loop.
                for qs in range(4):
                    qi = 4 * qb + qs
                    blk = pt[:, qi, qs * D:(qs + 1) * D]
                    nc.vector.tensor_mul(blk, blk, umask)
                return pt

            def emit_pv(h, qb, pt):
                """PV with ones column: out_aug[q,0:128]=O, [:,128]=row sums."""
                g = h // GROUP
                va = vas[g]
                ot = ost_pool.tile([D, 4, D], F32, tag="ot")
                for qs in range(4):
                    qi = 4 * qb + qs
                    oa = oa_pool.tile([D, D + 1], F32, tag="oa")
                    for ki in range(qi + 1):
                        nc.tensor.matmul(
                            oa, pt[:, ki, qs * D:(qs + 1) * D], va[:, ki, :],
                            start=(ki == 0), stop=(ki == qi),
                        )
                    rl = small.tile([D, 1], F32, tag="rl")
                    nc.vector.reciprocal(rl, oa[:, D:D + 1])
                    nc.vector.tensor_scalar_mul(ot[:, qs, :], oa[:, 0:D], rl)
                nc.gpsimd.dma_start(out=o_view[:, qb, :, h, :], in_=ot)

            # Software pipeline: emit PV one (h, qb) behind the score stage.
            prev = None
            for h in range(NQH):
                # whole-head Q^T: [d, q_global], plain load (host-transposed)
                if h == 0:
                    qt = qt0
                else:
                    qt = qt_pool.tile([D, S], BF16, tag="qt")
                    nc.sync.dma_start(out=qt, in_=q_ext[h * D:(h + 1) * D, :])
                for qb in range(NQB):
                    pt = emit_scores(h, qb, qt)
                    if prev is not None:
                        emit_pv(prev[0], prev[1], prev[2])
                    prev = (h, qb, pt)
            emit_pv(prev[0], prev[1], prev[2])
    nc.finalize()
    return nc


_NC_CACHE = None


def make_in_maps(inputs):
    bf16 = mybir.dt.np(BF16)
    query = np.asarray(inputs["query"], dtype=np.float32).astype(bf16)
    key = np.asarray(inputs["key"], dtype=np.float32).astype(bf16)
    value = np.asarray(inputs["value"], dtype=np.float32).astype(bf16)

    in_maps = []
    for c in range(8):
        b, hb = c // 4, c % 4
        in_maps.append({
            "queryT": np.ascontiguousarray(query[b, :, hb * 1024:(hb + 1) * 1024].T),
            "keyT": np.ascontiguousarray(key[b, :, hb * 256:(hb + 1) * 256].T),
            "value": np.ascontiguousarray(value[b, :, hb * 256:(hb + 1) * 256]),
        })
    return in_maps


def kernel(**inputs):
    global _NC_CACHE
    in_maps = make_in_maps(inputs)
    if _NC_CACHE is None:
        _NC_CACHE = build_nc()
    res = run_bass_kernel_spmd(_NC_CACHE, in_maps, list(range(8)))

    out = np.empty((2, 2048, 4096), dtype=np.float32)
    for c in range(8):
        b, hb = c // 4, c % 4
        out[b, :, hb * 1024:(hb + 1) * 1024] = res.results[c]["out"]
    return out


# revision 30
# speedup vs baseline: 1.0919x; 1.0919x over previous
"""GQA causal attention (B=2, S=2048, 32 Q heads / 8 KV heads, d=128) on 8 TRN2 cores.

Sharding: core c -> batch c//4, head-block c%4 (8 Q heads, 2 KV heads each).
Host side: inputs cast to bf16 and Q/K pre-transposed to (d, seq) layout, so
the device does no transposes and every DMA is a plain contiguous load.
Per-core kernel: S^T = K @ Q^T on the tensor engine (bf16, fp32 PSUM);
exp without max-subtraction (scores are O(6)); row sums via a ones-column
appended to V; causal handled by skipping k>q tiles, trimming diagonal-chunk
matmuls/exp to the valid q-range, and a triangular mask on diagonal blocks.
Softmax normalization and the output stay fp32.
"""

import sys

for p in ("/opt/trn_rl_repo", "/root/.axon_site/_ro/trn_rl_repo"):
    if p not in sys.path:
        sys.path.insert(0, p)

import numpy as np

import concourse.bass as bass
import concourse.mybir as mybir
import concourse.tile as tile
from concourse import bacc
from concourse.bass_utils import run_bass_kernel_spmd
from concourse.masks import make_upper_triangular

S = 2048            # sequence length
D = 128             # head size
NQH = 8             # query heads per core
NKVH = 2            # kv heads per core
GROUP = 4           # query heads per kv head
SCALE = 1.0 / np.sqrt(128.0)
NQT = S // D        # 16 q/k tiles of 128 rows
QB = 512            # q-block (4 tiles)
NQB = S // QB       # 4 q-blocks
F32 = mybir.dt.float32
BF16 = mybir.dt.bfloat16


def build_nc():
    nc = bacc.Bacc()
    # queryT/keyT are host-pretransposed to (d, seq); value is natural (seq, d)
    q_ext = nc.declare_dram_parameter("queryT", [NQH * D, S], BF16, isOutput=False)
    k_ext = nc.declare_dram_parameter("keyT", [NKVH * D, S], BF16, isOutput=False)
    v_ext = nc.declare_dram_parameter("value", [S, NKVH * D], BF16, isOutput=False)
    o_ext = nc.declare_dram_parameter("out", [S, NQH * D], F32, isOutput=True)

    with tile.TileContext(nc) as tc:
        with (
            tc.tile_pool(name="singles", bufs=1) as singles,
            tc.tile_pool(name="qt", bufs=3) as qt_pool,
            tc.tile_pool(name="pt", bufs=4) as pt_pool,
            tc.tile_pool(name="small", bufs=4) as small,
            tc.tile_pool(name="ost", bufs=4) as ost_pool,
            tc.tile_pool(name="st", bufs=2, space="PSUM") as st_pool,
            tc.tile_pool(name="oa", bufs=4, space="PSUM") as oa_pool,
        ):
            umask = singles.tile([D, D], BF16)  # umask[k, q] = 1.0 if k <= q else 0
            make_upper_triangular(nc, umask, val=1.0, diag=True)

            vv_view = v_ext[:].rearrange("(ki k) (g d) -> k ki g d", k=D, d=D)
            o_view = o_ext[:].rearrange("(qb qs r) (h d) -> r qb qs h d", r=D, qs=4, d=D)

            # K^T per kv head: [d, k_global], plain load (host-transposed).
            # First chunk of g=0 lands first so matmuls can start early.
            kts = []
            for g in range(NKVH):
                kt = singles.tile([D, S], BF16, tag=f"kt{g}")
                kts.append(kt)
            nc.sync.dma_start(out=kts[0][:, 0:QB], in_=k_ext[0:D, 0:QB])
            qt0 = qt_pool.tile([D, S], BF16, tag="qt")
            for c in range(4):
                nc.sync.dma_start(
                    out=qt0[:, c * QB:(c + 1) * QB],
                    in_=q_ext[0:D, c * QB:(c + 1) * QB],
                )
            for g in range(NKVH):
                for c in range(4):
                    if g == 0 and c == 0:
                        continue
                    nc.sync.dma_start(
                        out=kts[g][:, c * QB:(c + 1) * QB],
                        in_=k_ext[g * D:(g + 1) * D, c * QB:(c + 1) * QB],
                    )

            # V per kv head, natural layout + ones column: [k_off, ki, d|1].
            # Loaded via the gpsimd queue to keep the sync queue free for Q/K.
            vas = []
            for g in range(NKVH):
                va = singles.tile([D, NQT, D + 1], BF16, tag=f"va{g}")
                vas.append(va)
                nc.vector.memset(va[:, :, D], 1.0)
                nc.gpsimd.dma_start(out=va[:, :, 0:D], in_=vv_view[:, :, g, :])

            def emit_scores(h, qb, qt):
                """S^T matmuls + exp + diagonal mask for one (head, q-block).
                Returns the P^T tile."""
                g = h // GROUP
                kt = kts[g]
                nki = 4 * qb + 4  # causal: k tiles 0 .. 4qb+3
                qts = qt[:, qb * QB:(qb + 1) * QB]
                pt = pt_pool.tile([D, NQT, QB], BF16, tag="pt")
                CH = 2  # k-tiles per PSUM chunk (2 banks x 2 bufs + 4 oa = 8)
                for c in range((nki + CH - 1) // CH):
                    k0 = c * CH
                    nk = min(CH, nki - k0)
                    st = st_pool.tile([D, CH, QB], F32, tag="st")
                    for j in range(nk):
                        ki = k0 + j
                        s = ki - 4 * qb  # >=0 only in the 4 diagonal k-tiles
                        q0 = max(s, 0) * D  # valid q-range starts at tile s
                        nc.tensor.matmul(
                            st[:, j, q0:QB],
                            kt[:, ki * D:(ki + 1) * D], qts[:, q0:QB],
                            start=True, stop=True,
                        )
                    # exp only the valid q-range of the chunk (q >= the
                    # first k-tile's diagonal); unseen regions are never read
                    qc = max(k0 - 4 * qb, 0) * D
                    nc.scalar.activation(
                        pt[:, k0:k0 + nk, qc:QB], st[:, 0:nk, qc:QB],
                        mybir.ActivationFunctionType.Exp,
                        scale=float(SCALE),
                    )
                # causal fixup: mask the diagonal blocks (ki == qi).
                # Blocks with ki > qi are never read by the PV loop.
                for qs in range(4):
                    qi = 4 * qb + qs
                    blk = pt[:, qi, qs * D:(qs + 1) * D]
                    nc.vector.tensor_mul(blk, blk, umask)
                return pt

            def emit_pv(h, qb, pt):
                """PV with ones column: out_aug[q,0:128]=O, [:,128]=row sums."""
                g = h // GROUP
                va = vas[g]
                ot = ost_pool.tile([D, 4, D], F32, tag="ot")
                for qs in range(4):
                    qi = 4 * qb + qs
                    oa = oa_pool.tile([D, D + 1], F32, tag="oa")
                    for ki in range(qi + 1):
                        nc.tensor.matmul(
                            oa, pt[:, ki, qs * D:(qs + 1) * D], va[:, ki, :],
                            start=(ki == 0), stop=(ki == qi),
                        )
                    rl = small.tile([D, 1], F32, tag="rl")
                    nc.vector.reciprocal(rl, oa[:, D:D + 1])
                    nc.vector.tensor_scalar_mul(ot[:, qs, :], oa[:, 0:D], rl)
                nc.gpsimd.dma_start(out=o_view[:, qb, :, h, :], in_=ot)

            # Software pipeline: emit PV one (h, qb) behind the score stage.
            prev = None
            for h in range(NQH):
                # whole-head Q^T: [d, q_global], plain load (host-transposed)
                if h == 0:
                    qt = qt0
                else:
                    qt = qt_pool.tile([D, S], BF16, tag="qt")
                    nc.sync.dma_start(out=qt, in_=q_ext[h * D:(h + 1) * D, :])
                for qb in range(NQB):
                    pt = emit_scores(h, qb, qt)
                    if prev is not None:
                        emit_pv(prev[0], prev[1], prev[2])
                    prev = (h, qb, pt)
            emit_pv(prev[0], prev[1], prev[2])
    nc.finalize()
    return nc


_NC_CACHE = None


def make_in_maps(inputs):
    bf16 = mybir.dt.np(BF16)
    query = np.asarray(inputs["query"], dtype=np.float32).astype(bf16)
    key = np.asarray(inputs["key"], dtype=np.float32).astype(bf16)
    value = np.asarray(inputs["value"], dtype=np.float32).astype(bf16)

    in_maps = []
    for c in range(8):
        b, hb = c // 4, c % 4
        in_maps.append({
            "queryT": np.ascontiguousarray(query[b, :, hb * 1024:(hb + 1) * 1024].T),
            "keyT": np.ascontiguousarray(key[b, :, hb * 256:(hb + 1) * 256].T),
            "value": np.ascontiguousarray(value[b, :, hb * 256:(hb + 1) * 256]),
        })
    return in_maps


def kernel(**inputs):
    global _NC_CACHE
    in_maps = make_in_maps(inputs)
    if _NC_CACHE is None:
        _NC_CACHE = build_nc()
    res = run_bass_kernel_spmd(_NC_CACHE, in_maps, list(range(8)))

    out = np.empty((2, 2048, 4096), dtype=np.float32)
    for c in range(8):
        b, hb = c // 4, c % 4
        out[b, :, hb * 1024:(hb + 1) * 1024] = res.results[c]["out"]
    return out
